# revision 7
# baseline (speedup 1.0000x reference)
"""Negative-sampling loss kernel for Trainium2 (8 NeuronCores, SPMD).

Strategy
--------
Data-parallel over batch B=262144 across 8 cores (32768 items each).
The computation is two scalar losses derived from per-item dot products
between gathered embedding rows:

    pos:  s_b = input_emb[t_b] . output_emb[c_b]          (B scores)
    neg:  s_bj = input_emb[t_b] . output_emb[n_bj]        (B*5 scores)

Each (t, other) pair is gathered independently on-device with the
custom `dma_gather` instruction (256B bf16 rows).  `dma_gather` uses
int16 indices, so each pair is bucketed by the 32768-row vocab windows
of both its words (16 groups); every gather instruction then has a
static window base and in-window (<32768) indices.  pos and neg pairs
share one bucketing (the pos/neg split is host-side bookkeeping), the
two gathers of a tile run on different SWDGE queues, index tiles are
DMA'd in 32-tile strips, and the per-pair dot products run as fused
tensor_tensor_reduce ops (one per 128-pair block).  The final
softplus + mean runs on the host in float64.

Tables are converted to bf16 on the host: score rel-error ~1% per item
averages out over 262144 items (final loss rel-error ~1e-7).
"""

import os
from contextlib import ExitStack

import numpy as np
import ml_dtypes

USE_TTR = os.environ.get("K_TTR", "0") == "1"
USE_QROT = os.environ.get("K_QROT", "1") == "1"

import concourse.bass as bass
import concourse.bacc as bacc
import concourse.mybir as mybir
import concourse.tile as tile
from concourse.bass_utils import run_bass_kernel_spmd

VOCAB = 100000
D = 128
B = 262144
NEG = 5
NCORES = 8
BPC = B // NCORES          # 32768 items per core
WIN = 32768                # dma_gather int16 index window
NWIN = (VOCAB + WIN - 1) // WIN   # 4
CHUNK = 1024               # gathered rows per dma_gather (HW ring cap)
KPT = CHUNK // 128         # 8 score columns per tile
NQ = 4                     # SWDGE queues
STRIP = 32                 # tiles per idx strip DMA

_cache = {}


def _build_nc(n_tiles, tile_meta):
    """One SPMD program: n_tiles tiles, tile_meta[i] = (t_win, p_win)."""
    n_strips = (n_tiles + STRIP - 1) // STRIP
    nc = bacc.Bacc(
        "TRN2",
        target_bir_lowering=False,
        debug=False,
        enable_asserts=False,
        num_swdge_queues=NQ,
    )
    in_emb = nc.dram_tensor("in_emb", [VOCAB, D], mybir.dt.bfloat16, kind="ExternalInput")
    out_emb = nc.dram_tensor("out_emb", [VOCAB, D], mybir.dt.bfloat16, kind="ExternalInput")
    # idx strips: per strip, per partition: STRIP tiles x (t,p) x 64 int16
    idx = nc.dram_tensor("idx", [n_strips, 128, STRIP * 2 * (CHUNK // 16)],
                         mybir.dt.int16, kind="ExternalInput")
    sc_out = nc.dram_tensor("sc_out", [128, n_tiles * KPT], mybir.dt.float32, kind="ExternalOutput")

    W = CHUNK // 16  # 64 int16 words per tile per side per partition
    with tile.TileContext(nc) as tc, ExitStack() as ctx:
        idxp = ctx.enter_context(tc.tile_pool(name="idx", bufs=int(os.environ.get("K_IBUFS", "2"))))
        gatp = ctx.enter_context(tc.tile_pool(name="gat", bufs=int(os.environ.get("K_BUFS", "12"))))
        scrp = ctx.enter_context(tc.tile_pool(name="scr", bufs=4))
        scp = ctx.enter_context(tc.tile_pool(name="sc", bufs=1))

        sc_all = scp.tile([128, n_tiles * KPT], mybir.dt.float32)
        strip_tile = None
        for t in range(n_tiles):
            s, k = divmod(t, STRIP)
            if k == 0:
                strip_tile = idxp.tile([128, STRIP * 2 * W], mybir.dt.int16, tag="strip")
                nc.sync.dma_start(out=strip_tile[:], in_=idx[s])
            t_win, p_win = tile_meta[t]
            ti = strip_tile[:, (2 * k) * W:(2 * k + 1) * W]
            pi = strip_tile[:, (2 * k + 1) * W:(2 * k + 2) * W]

            tt = gatp.tile([128, KPT * D], mybir.dt.bfloat16, tag="tt")
            pt = gatp.tile([128, KPT * D], mybir.dt.bfloat16, tag="pt")
            q0 = (2 * t) % NQ if USE_QROT else t % NQ
            q1 = (2 * t + 1) % NQ if USE_QROT else t % NQ
            nc.gpsimd.dma_gather(
                tt[:].rearrange("p (g d) -> p g d", d=D),
                in_emb[t_win * WIN:, :],
                ti, CHUNK, CHUNK, D, elem_step=D,
                queue_num=q0,
            )
            nc.gpsimd.dma_gather(
                pt[:].rearrange("p (g d) -> p g d", d=D),
                out_emb[p_win * WIN:, :],
                pi, CHUNK, CHUNK, D, elem_step=D,
                queue_num=q1,
            )
            if USE_TTR:
                scr = scrp.tile([128, KPT * D], mybir.dt.bfloat16, tag="scr")
                for b in range(KPT):
                    nc.vector.tensor_tensor_reduce(
                        out=scr[:, b * D:(b + 1) * D],
                        in0=pt[:, b * D:(b + 1) * D],
                        in1=tt[:, b * D:(b + 1) * D],
                        scale=1.0,
                        scalar=0.0,
                        op0=mybir.AluOpType.mult,
                        op1=mybir.AluOpType.add,
                        accum_out=sc_all[:, t * KPT + b:t * KPT + b + 1],
                    )
            else:
                tt3 = tt[:].rearrange("p (g d) -> p g d", d=D)
                pt3 = pt[:].rearrange("p (g d) -> p g d", d=D)
                scr = scrp.tile([128, KPT * D], mybir.dt.bfloat16, tag="scr")
                scr3 = scr[:].rearrange("p (g d) -> p g d", d=D)
                nc.vector.tensor_tensor(out=scr3, in0=pt3, in1=tt3, op=mybir.AluOpType.mult)
                nc.vector.tensor_reduce(
                    out=sc_all[:, t * KPT:(t + 1) * KPT],
                    in_=scr3, axis=mybir.AxisListType.X, op=mybir.AluOpType.add)
        nc.sync.dma_start(out=sc_out[:], in_=sc_all[:])
    nc.finalize()
    return nc


def _plan_and_pack(target_words, context_words, negative_words):
    """Bucket all (t, other) pairs per core by (t_win, o_win); build the
    shared tile schedule and per-core packed index strips."""
    t_w = np.asarray(target_words).astype(np.int64).reshape(NCORES, BPC)
    c_w = np.asarray(context_words).astype(np.int64).reshape(NCORES, BPC)
    n_w = np.asarray(negative_words).astype(np.int64).reshape(NCORES, BPC, NEG)

    # flatten to per-core pair lists: (t, o, is_pos)
    PPC = BPC * (1 + NEG)   # pairs per core
    t_all = np.concatenate([t_w[:, :, None], np.repeat(t_w[:, :, None], NEG, axis=2)],
                           axis=2).reshape(NCORES, PPC)
    o_all = np.concatenate([c_w[:, :, None], n_w], axis=2).reshape(NCORES, PPC)
    pos_all = np.zeros((NCORES, PPC), bool)
    pos_all.reshape(NCORES, BPC, 1 + NEG)[:, :, 0] = True

    NG = NWIN * NWIN
    key = (t_all // WIN) * NWIN + (o_all // WIN)   # [NCORES, PPC]
    cnt = np.zeros((NCORES, NG), np.int64)
    for c in range(NCORES):
        cnt[c] = np.bincount(key[c], minlength=NG)
    g_tiles = np.ceil(cnt.max(axis=0) / CHUNK).astype(int)
    n_tiles = int(g_tiles.sum())
    g_tile_start = np.concatenate([[0], np.cumsum(g_tiles)])[:-1]

    tile_meta = []
    for g in range(NG):
        tile_meta += [(g // NWIN, g % NWIN)] * g_tiles[g]

    tidx_all = np.zeros((NCORES, n_tiles, CHUNK), np.int16)
    pidx_all = np.zeros((NCORES, n_tiles, CHUNK), np.int16)
    valid_all = np.zeros((NCORES, n_tiles, CHUNK), bool)
    ispos_all = np.zeros((NCORES, n_tiles, CHUNK), bool)
    for c in range(NCORES):
        order = np.argsort(key[c], kind="stable")
        ks = key[c][order]
        tw = t_all[c][order]
        ow = o_all[c][order]
        ps = pos_all[c][order]
        bounds = np.searchsorted(ks, np.arange(NG + 1))
        for g in range(NG):
            lo, hi = bounds[g], bounds[g + 1]
            if hi == lo:
                continue
            cnt_g = hi - lo
            sl = np.s_[g_tile_start[g]:g_tile_start[g] + g_tiles[g]]
            tl = tidx_all[c, sl].reshape(-1)
            pl = pidx_all[c, sl].reshape(-1)
            vl = valid_all[c, sl].reshape(-1)
            il = ispos_all[c, sl].reshape(-1)
            tl[:cnt_g] = (tw[lo:hi] - (g // NWIN) * WIN).astype(np.int16)
            pl[:cnt_g] = (ow[lo:hi] - (g % NWIN) * WIN).astype(np.int16)
            vl[:cnt_g] = True
            il[:cnt_g] = ps[lo:hi]

    # pack idx strips: [n_strips, 128, STRIP*2*64] int16 per core
    n_strips = (n_tiles + STRIP - 1) // STRIP
    n_pad = n_strips * STRIP

    def wrap16(v):  # [n_tiles, CHUNK] -> [n_tiles, 128, CHUNK//16]
        w = v.reshape(v.shape[0], CHUNK // 16, 16).transpose(0, 2, 1)
        return np.tile(w, (1, 8, 1))

    per_core = []
    for c in range(NCORES):
        tw16 = wrap16(tidx_all[c])   # [n_tiles, 128, 64]
        pw16 = wrap16(pidx_all[c])
        both = np.zeros((n_pad, 2, 128, CHUNK // 16), np.int16)
        both[:n_tiles, 0] = tw16
        both[:n_tiles, 1] = pw16
        # -> [n_strips, 128, STRIP*2*64]
        strips = (both.reshape(n_strips, STRIP, 2, 128, CHUNK // 16)
                  .transpose(0, 3, 1, 2, 4)
                  .reshape(n_strips, 128, STRIP * 2 * (CHUNK // 16)).copy())
        per_core.append(strips)
    return tile_meta, per_core, valid_all, ispos_all, n_tiles


def _unpack(sc_outs, valid_all, ispos_all, n_tiles):
    """sc_outs: per-core [128, n_tiles*KPT] float32; slot s of tile t is at
    sc[s % 128, t*KPT + s//128]."""
    pos_sum = 0.0
    neg_sum = 0.0
    for c in range(len(sc_outs)):
        sc = np.asarray(sc_outs[c]).astype(np.float64)
        sc = sc.reshape(128, n_tiles, KPT).transpose(1, 2, 0).reshape(n_tiles, CHUNK)
        v = valid_all[c]
        p = ispos_all[c]
        pos_scores = sc[v & p]
        neg_scores = sc[v & ~p]
        pos_sum += np.logaddexp(0.0, -pos_scores).sum()
        neg_sum += np.logaddexp(0.0, neg_scores).sum()
    positive_loss = np.float32(pos_sum / B)
    negative_loss = np.float32(neg_sum / (B * NEG))
    return positive_loss, negative_loss


def kernel(target_words, context_words, negative_words, input_emb, output_emb,
           _want_results=False, _trace=False):
    input_emb = np.asarray(input_emb)
    output_emb = np.asarray(output_emb)
    in_bf = input_emb.astype(ml_dtypes.bfloat16)
    out_bf = output_emb.astype(ml_dtypes.bfloat16)

    tile_meta, per_core, valid_all, ispos_all, n_tiles = _plan_and_pack(
        target_words, context_words, negative_words)

    key = (n_tiles, tuple(tile_meta))
    if key not in _cache:
        _cache[key] = _build_nc(n_tiles, tile_meta)
    nc = _cache[key]

    in_maps = []
    for c in range(NCORES):
        in_maps.append({
            "in_emb": in_bf,
            "out_emb": out_bf,
            "idx": per_core[c],
        })
    br = run_bass_kernel_spmd(nc, in_maps, core_ids=list(range(NCORES)),
                              trace=_trace)

    positive_loss, negative_loss = _unpack(
        [br.results[c]["sc_out"] for c in range(NCORES)],
        valid_all, ispos_all, n_tiles)
    if _want_results:
        return (positive_loss, negative_loss), br
    return (positive_loss, negative_loss)
